# revision 64
# baseline (speedup 1.0000x reference)
"""CWT scalogram layer on Trainium2 (Bass/Tile), 8-core data-parallel.

Pipeline per core (16 batch rows):
  1. fold x into X[q, col] layout (q = position mod 128) via PE transposes
  2. CWT: per scale, banded-Toeplitz matmuls accumulate in PSUM; Abs-evict
     to a bf16 scalogram tile ScAll[q, scale, col]
  3. time resize 2048->224 as accumulating matmuls over time-blocks
  4. PE transpose to put scales on partitions; scale resize 64->224
  5. per-image min/max, fused (img - mn)/(mx - mn + eps) then channel
     mean/std affine during PSUM->SBUF eviction; DMA out
"""

import sys
import numpy as np

for _p in ("/opt/trn_rl_repo",):
    if _p not in sys.path:
        sys.path.insert(0, _p)

import ml_dtypes

# ---------------------------------------------------------------------------
# problem constants (hardcoded per spec)
# ---------------------------------------------------------------------------
B_FULL = 128
N_CORES = 8
B_LOC = B_FULL // N_CORES          # 16
N = 2048
SCALES = 64
IMG = 224
PRECISION = 10
P = 128
NT = N // P                        # 16 time blocks
GAP = 5                            # zero blocks between batches (>= max |d|)
STRIDE = NT + GAP                  # 21 columns per batch in packed layout
COLS = B_LOC * STRIDE + GAP        # 341 packed columns (incl. tail gap)
XW = COLS + 2 * GAP + 1            # 352, X buffer with +-5 margin (even size)
IC = IMG // 2                      # 112, i/j chunk


# ---------------------------------------------------------------------------
# numpy constant builders
# ---------------------------------------------------------------------------
def _int_psi_morlet():
    M = 2 ** PRECISION
    xs = np.linspace(-8.0, 8.0, M)
    psi = np.exp(-xs ** 2 / 2.0) * np.cos(5.0 * xs)
    step = xs[1] - xs[0]
    return np.cumsum(psi) * step, step


def _scale_kernels():
    int_psi, step = _int_psi_morlet()
    kernels = []
    for s in range(1, SCALES + 1):
        j = np.floor(np.arange(16 * s + 1) / (s * step)).astype(np.int64)
        j = j[j < int_psi.size]
        kernels.append(int_psi[j].astype(np.float32))
    return kernels


def _wfull():
    """W[s-1, delta+513] so that coef[b,s,n] = sum_d W[s-1,d+513] x[b,n+d]."""
    kernels = _scale_kernels()
    W = np.zeros((SCALES, 1026), dtype=np.float64)
    for s in range(1, SCALES + 1):
        k = kernels[s - 1].astype(np.float64)
        dk = np.concatenate([[-k[0]], k[:-1] - k[1:], [k[-1]]])  # len 16s+2
        deltas = np.arange(-8 * s - 1, 8 * s + 1)
        W[s - 1, deltas + 513] = -np.sqrt(float(s)) * dk
    return W


def _d_range(s):
    d_min = -((8 * s + P) // P)
    d_max = (8 * s + P - 1) // P
    return list(range(d_min, d_max + 1))


def _build_cwt_weights():
    """Stationary matrices A_{s,d}[q, m] = W[s, q - m + 128 d + 513]."""
    W = _wfull()
    q = np.arange(P)[:, None]
    m = np.arange(P)[None, :]
    mats = []          # list of (s, d, [128,128] float32)
    d_lists = []
    for s in range(1, SCALES + 1):
        ds_ = _d_range(s)
        d_lists.append(ds_)
        row = W[s - 1]
        for d in ds_:
            idx = q - m + P * d + 513
            ok = (idx >= 0) & (idx <= 1025)
            a = np.where(ok, row[np.clip(idx, 0, 1025)], 0.0)
            mats.append(a.astype(np.float32))
    return np.stack(mats), d_lists   # [TOTD, 128, 128]


def _keys_cubic(x):
    out = ((1.5 * x - 2.5) * x) * x + 1.0
    out = np.where(x >= 1.0, ((-0.5 * x + 2.5) * x - 4.0) * x + 2.0, out)
    return np.where(x >= 2.0, 0.0, out)


def _resize_wmat(n_in, n_out):
    """Replica of jax.image.resize(method='cubic') weights: [n_in, n_out]."""
    scale = np.float32(n_out) / np.float32(n_in)
    inv_scale = np.float32(1.0) / scale
    kernel_scale = max(inv_scale, np.float32(1.0))
    sample_f = ((np.arange(n_out, dtype=np.float32) + 0.5) * inv_scale
                - np.float32(0.5))
    x = np.abs(sample_f[None, :]
               - np.arange(n_in, dtype=np.float32)[:, None]) / kernel_scale
    w = _keys_cubic(x.astype(np.float32)).astype(np.float32)
    total = w.sum(axis=0, keepdims=True)
    w = np.where(np.abs(total) > 1000.0 * np.finfo(np.float32).eps,
                 w / np.where(total != 0, total, 1), 0.0)
    ok = (sample_f >= -0.5) & (sample_f <= n_in - 0.5)
    return np.where(ok[None, :], w, 0.0).astype(np.float32)


def build_consts():
    A, d_lists = _build_cwt_weights()          # [TOTD,128,128]
    totd = A.shape[0]
    # DRAM layout [q, jidx, m] so each per-scale DMA reads contiguous chunks
    W_ALL = np.ascontiguousarray(A.transpose(1, 0, 2)).reshape(P, totd * P)
    wt = _resize_wmat(N, IMG)                  # [2048, 224]
    MT = np.ascontiguousarray(
        wt.reshape(NT, P, IMG).transpose(1, 0, 2)).reshape(P, NT * IMG)
    tmask = [[bool(np.any(wt[t * P:(t + 1) * P, jc * IC:(jc + 1) * IC]))
              for jc in range(2)] for t in range(NT)]
    MS = _resize_wmat(SCALES, IMG)             # [64, 224]
    eye112 = np.eye(IC, dtype=np.float32)
    bf16 = ml_dtypes.bfloat16
    return dict(
        W_ALL=W_ALL.astype(bf16),
        MT=MT.astype(bf16),
        MS=MS.astype(bf16),
        eye112=eye112,
        d_lists=d_lists,
        tmask=tmask,
        totd=totd,
    )


# ---------------------------------------------------------------------------
# golden numpy model of the device computation (for offline validation)
# ---------------------------------------------------------------------------
def golden(x, mean, std, quant=True):
    """Emulate the device computation in numpy. x: (B_LOC, N) float32."""
    c = build_consts()
    bf16 = ml_dtypes.bfloat16

    def q16(a):
        return a.astype(bf16).astype(np.float32) if quant else a.astype(np.float32)

    A, d_lists = _build_cwt_weights()
    xq = q16(x)
    # X buffer [128, XW]
    X = np.zeros((P, XW), dtype=np.float32)
    for b in range(B_LOC):
        for t in range(NT):
            X[:, GAP + STRIDE * b + t] = xq[b, t * P:(t + 1) * P]
    X = q16(X)
    # CWT
    scall = np.zeros((P, SCALES, COLS), dtype=np.float32)
    j = 0
    for si in range(SCALES):
        ps = np.zeros((P, COLS), dtype=np.float32)
        for d in d_lists[si]:
            a = q16(A[j]); j += 1
            ps += a.T @ X[:, GAP + d: GAP + d + COLS]
        scall[:, si, :] = np.abs(ps)
    scall = q16(scall)
    # time resize
    wt = q16(_resize_wmat(N, IMG))
    img1 = np.zeros((IMG, B_LOC, SCALES), dtype=np.float32)
    for t in range(NT):
        blk = wt[t * P:(t + 1) * P, :]             # [128, 224]
        sc = scall[:, :, [STRIDE * b + t for b in range(B_LOC)]]  # [128,S,B]
        img1 += np.einsum('qj,qsb->jbs', blk, sc)
    img1 = q16(img1)
    # scale resize
    ws = q16(_resize_wmat(SCALES, IMG))
    img = q16(np.einsum('si,jbs->bij', ws, img1))  # [B, 224i, 224j]
    mn = img.min(axis=(1, 2), keepdims=True)
    mx = img.max(axis=(1, 2), keepdims=True)
    r = 1.0 / (mx - mn + 1e-8)
    imgn = (img - mn) * r
    mean = np.asarray(mean, np.float32).reshape(3)
    std = np.asarray(std, np.float32).reshape(3)
    out = (imgn[:, None] - mean[None, :, None, None]) / std[None, :, None, None]
    return out.astype(np.float32)


# ---------------------------------------------------------------------------
# bass kernel
# ---------------------------------------------------------------------------
_NC_CACHE = {}


def _build_nc(split_waits=True):
    import concourse.bass as bass
    import concourse.mybir as mybir
    import concourse.tile as tile
    from concourse.tile_rust import add_dep_helper

    c = build_consts()
    d_lists = c["d_lists"]
    tmask = c["tmask"]
    F32 = mybir.dt.float32
    BF16 = mybir.dt.bfloat16
    AF = mybir.ActivationFunctionType
    ALU = mybir.AluOpType
    AX = mybir.AxisListType

    nc = bass.Bass()

    x_in = nc.dram_tensor("x", [B_LOC, N], F32, kind="ExternalInput")
    mean_in = nc.dram_tensor("mean", [1, 3, 1, 1], F32, kind="ExternalInput")
    std_in = nc.dram_tensor("std", [1, 3, 1, 1], F32, kind="ExternalInput")
    out_d = nc.dram_tensor("out", [B_LOC, 3, IMG, IMG], F32,
                           kind="ExternalOutput")

    w_all_d = nc.inline_tensor(np.asarray(c["W_ALL"]), name="w_all")
    mt_d = nc.inline_tensor(np.asarray(c["MT"]), name="mt_w")
    ms_d = nc.inline_tensor(np.asarray(c["MS"]), name="ms_w")
    eye_d = nc.inline_tensor(c["eye112"], name="eye112")

    with tile.TileContext(nc) as tc:
        with tc.tile_pool(name="const", bufs=1) as cpool:
            x_sb = cpool.tile([B_LOC, N], F32)
            nc.sync.dma_start(x_sb[:], x_in[:])
            eye = cpool.tile([IC, IC], F32)
            nc.sync.dma_start(eye[:], eye_d[:])
            mt_sb = cpool.tile([P, NT, IMG], BF16)
            nc.sync.dma_start(
                mt_sb[:], mt_d[:].rearrange("q (t j) -> q t j", j=IMG))
            # Ms weights duplicated in both partition halves so the
            # scale-resize lhsT base partition can match its rhs
            ms_sb = cpool.tile([P, IMG], BF16)
            nc.sync.dma_start(ms_sb[0:SCALES, :], ms_d[:])
            nc.sync.dma_start(ms_sb[SCALES:P, :], ms_d[:])
            ms_row = cpool.tile([1, 2, 3], F32)
            nc.sync.dma_start(ms_row[:, 0, :],
                              mean_in[:].rearrange("a b c d -> a (b c d)"))
            nc.sync.dma_start(ms_row[:, 1, :],
                              std_in[:].rearrange("a b c d -> a (b c d)"))
            ones_sb = cpool.tile([1, IC], F32)
            nc.vector.memset(ones_sb[:], 1.0)

            xbuf = cpool.tile([P, XW], BF16)
            nc.scalar.memzero(xbuf[:])
            # [q, t, s, b]: the time-resize moving operand (fixed t, all
            # (s, b)) is then fully contiguous, and the CWT eviction writes
            # 16-element contiguous runs (b innermost)
            scall = cpool.tile([P, NT, SCALES, B_LOC], BF16)
            img1 = cpool.tile([IC, 2, B_LOC, 2, SCALES // 2], F32)
            t_full = cpool.tile([P, 2, B_LOC // 2, IC], BF16)
            # min/max scalar chain state, duplicated per 8-batch wave so the
            # second wave's reductions overlap the first wave's normalize
            NW, WB = 2, B_LOC // 2
            mm_sb = [cpool.tile([IC, 2, 32], F32, name=f"mmsb{w}")
                     for w in range(NW)]
            mm_r = [cpool.tile([64, 1], F32, name=f"mmr{w}")
                    for w in range(NW)]
            row = [cpool.tile([1, 64], F32, name=f"row{w}") for w in range(NW)]
            sc_rng = cpool.tile([1, B_LOC], F32)
            sc_r = cpool.tile([1, B_LOC], F32)
            sc_b0 = cpool.tile([1, B_LOC], F32)
            inv_std = cpool.tile([1, 3], F32)
            ninv_std = cpool.tile([1, 3], F32)
            scrow = cpool.tile([1, NW, 2, 3, WB], F32)
            scb = [cpool.tile([IC, 2, 3, WB], F32, name=f"scb{w}")
                   for w in range(NW)]
            # unused padding slots flow through the PE transposes; zero them
            for w in range(NW):
                nc.vector.memset(mm_sb[w][:], 0.0)
                nc.vector.memset(mm_r[w][:], 0.0)

            # stage 1: fold x into xbuf via PE transposes -------------------
            # all 16 transposes target disjoint column ranges of ONE psum
            # bank (start=True only clears has_written bits, not data), so
            # a single merged eviction suffices.
            xb_v = xbuf[:, GAP:GAP + B_LOC * STRIDE].rearrange(
                "q (b u) -> q b u", u=STRIDE)
            with tc.tile_pool(name="pfold", bufs=1, space="PSUM") as pfold:
                pt = pfold.tile([P, NT, B_LOC], F32)
                for t in range(NT):
                    nc.tensor.transpose(
                        pt[:, t, :], x_sb[:, t * P:(t + 1) * P],
                        eye[:B_LOC, :B_LOC])
                nc.scalar.activation(
                    xb_v[:, :, 0:NT].rearrange("q b u -> q u b"),
                    pt[:], AF.Copy)

            # stage 2+3: CWT scales + time-resize interleaved ---------------
            # weight DMAs grouped several scales at a time: bigger transfers
            # and one DMA semaphore per group (first group small so the PE
            # can start early)
            GRPS = [list(range(*r)) for r in
                    [(0, 2), (2, 8), (8, 16), (16, 24), (24, 32),
                     (32, 40), (40, 48), (48, 56), (56, 64)]]
            n_grp = len(GRPS)
            grp_nd = [sum(len(d_lists[si]) for si in g) for g in GRPS]
            nd_max = max(grp_nd)

            with (
                tc.tile_pool(name="pcwt", bufs=4, space="PSUM") as pcwt,
                tc.tile_pool(name="wpool", bufs=2) as wpool,
                tc.tile_pool(name="ptres", bufs=1, space="PSUM") as ptres,
            ):
                w_v = w_all_d[:].rearrange("q (j m) -> q j m", m=P)
                grp_off = [sum(grp_nd[:g]) for g in range(n_grp)]

                def cwt_group(g):
                    wt_t = wpool.tile([P, nd_max, P], BF16, tag="wt")
                    nc.sync.dma_start(
                        wt_t[:, 0:grp_nd[g], :],
                        w_v[:, grp_off[g]:grp_off[g] + grp_nd[g], :])
                    k0 = 0
                    for si in GRPS[g]:
                        ds_ = d_lists[si]
                        ps = pcwt.tile([P, COLS], F32, tag="cwtps")
                        for k, d in enumerate(ds_):
                            nc.tensor.matmul(
                                ps[:], wt_t[:, k0 + k, :],
                                xbuf[:, GAP + d: GAP + d + COLS],
                                start=(k == 0), stop=(k == len(ds_) - 1))
                        k0 += len(ds_)
                        src = ps[:, 0:B_LOC * STRIDE].rearrange(
                            "p (b u) -> p u b", u=STRIDE)[:, 0:NT, :]
                        nc.scalar.activation(scall[:, :, si, :], src, AF.Abs)

                # time-resize per scale-half, interleaved into the CWT
                # stream; t outer with the two jc psum banks round-robined
                # so consecutive matmuls never accumulate into the same bank
                sc_flat = scall[:].rearrange("q t s b -> q (t s b)")

                def tres_half(h):
                    ps2 = [ptres.tile([IC, 512], F32, name=f"tres{h}{jc}")
                           for jc in range(2)]
                    kidx = [0, 0]
                    n_t = [sum(1 for tt in range(NT) if tmask[tt][jc])
                           for jc in range(2)]
                    for t in range(NT):
                        for jc in range(2):
                            if not tmask[t][jc]:
                                continue
                            off = (t * SCALES + h * 32) * B_LOC
                            nc.tensor.matmul(
                                ps2[jc],
                                mt_sb[:, t, jc * IC:(jc + 1) * IC],
                                sc_flat[:, off:off + 512],
                                start=(kidx[jc] == 0),
                                stop=(kidx[jc] == n_t[jc] - 1))
                            kidx[jc] += 1
                    for jc in range(2):
                        # psum cols are (s, b); reorder to (b, s) on the way
                        # out so the transpose blocks stay (b-pair, h, s)
                        nc.scalar.activation(
                            img1[:, jc, :, h, :],
                            ps2[jc][:].rearrange("p (s b) -> p b s",
                                                 b=B_LOC), AF.Copy)

                for g in range(5):
                    cwt_group(g)
                tres_half(0)
                for g in range(5, n_grp):
                    cwt_group(g)
                tres_half(1)

            # stage 4: transpose + scale-resize + minmax --------------------
            with (
                tc.tile_pool(name="ptp", bufs=3, space="PSUM") as ptp,
                tc.tile_pool(name="pimg", bufs=3, space="PSUM") as pimg,
                tc.tile_pool(name="pmisc", bufs=2, space="PSUM") as pmisc,
                tc.tile_pool(name="imgsb", bufs=B_LOC) as imgsb_pool,
            ):
                for jc in range(2):
                    for k in range(B_LOC // 2):
                        pt = ptp.tile([P, IC], F32, tag="tp")
                        nc.tensor.transpose(
                            pt[:], img1[:, jc, 2 * k:2 * k + 2, :, :], eye[:])
                        nc.scalar.activation(t_full[:, jc, k, :], pt[:],
                                             AF.Copy)

                def minmax_chain(w):
                    """min/max partials of wave w -> per-image affine scb."""
                    pt1 = ptp.tile([P, IC], F32, tag="tp", name=f"pt1{w}")
                    nc.tensor.transpose(
                        pt1[0:64, :],
                        mm_sb[w][:].rearrange("p a b -> p (a b)"), eye[:])
                    nc.vector.tensor_reduce(mm_r[w][0:WB, :], pt1[0:WB, :],
                                            AX.X, ALU.min)
                    nc.vector.tensor_reduce(mm_r[w][32:32 + WB, :],
                                            pt1[32:32 + WB, :],
                                            AX.X, ALU.max)
                    pt2 = pmisc.tile([P, IC], F32, tag="misc", name=f"pt2{w}")
                    nc.tensor.transpose(pt2[0:1, 0:64], mm_r[w][:],
                                        eye[0:64, 0:64])
                    nc.vector.tensor_copy(row[w][:], pt2[0:1, 0:64])

                    rng = sc_rng[:, w * WB:(w + 1) * WB]
                    r_ = sc_r[:, w * WB:(w + 1) * WB]
                    b0 = sc_b0[:, w * WB:(w + 1) * WB]
                    nc.vector.tensor_tensor(rng, row[w][:, 32:32 + WB],
                                            row[w][:, 0:WB], ALU.subtract)
                    nc.vector.tensor_scalar_add(rng, rng, 1e-8)
                    nc.vector.reciprocal(r_, rng)
                    nc.vector.tensor_tensor(b0, row[w][:, 0:WB], r_,
                                            ALU.mult)
                    if w == 0:
                        nc.vector.reciprocal(inv_std[:], ms_row[:, 1, :])
                        nc.vector.tensor_scalar_mul(ninv_std[:], inv_std[:],
                                                    -1.0)
                    for ch in range(3):
                        nc.vector.tensor_scalar(
                            scrow[:, w, 0, ch, :], r_,
                            inv_std[:, ch:ch + 1], None, ALU.mult)
                        nc.vector.tensor_scalar(
                            scrow[:, w, 1, ch, :], b0,
                            ms_row[:, 0, ch:ch + 1], ninv_std[:, ch:ch + 1],
                            ALU.add, ALU.mult)
                    pbc = pmisc.tile([P, IC], F32, tag="misc", name=f"pbc{w}")
                    nc.tensor.matmul(
                        pbc[0:IC, 0:6 * WB], ones_sb[:],
                        scrow[:, w].rearrange("p a c b -> p (a c b)"),
                        start=True, stop=True)
                    nc.scalar.activation(
                        scb[w][:].rearrange("p a c b -> p (a c b)"),
                        pbc[0:IC, 0:6 * WB], AF.Copy)

                img_sb = []
                with tc.tile_pool(name="outp", bufs=4) as outp:
                    for b in range(B_LOC):
                        k, bl = b // 2, b % 2
                        tsrc = t_full[64 * bl:64 * (bl + 1)]
                        ms_half = ms_sb[64 * bl:64 * (bl + 1)]
                        isb = imgsb_pool.tile([IC, 2, 2, IC], BF16,
                                              tag="imgsb")
                        img_sb.append(isb)
                        # all 4 (ic, jc) chunks go into ONE psum bank;
                        # single merged eviction + one min/max pass each
                        pi = pimg.tile([IC, 2, 2, IC], F32, tag="img")
                        for ic in range(2):
                            for jc in range(2):
                                nc.tensor.matmul(
                                    pi[:, ic, jc, :],
                                    ms_half[:, ic * IC:(ic + 1) * IC],
                                    tsrc[:, jc, k, :], start=True, stop=True)
                        nc.scalar.activation(isb[:], pi[:], AF.Copy)
                        w, bw = b // WB, b % WB
                        flat = isb[:].rearrange("p a b j -> p (a b j)")
                        nc.vector.tensor_reduce(mm_sb[w][:, 0, bw:bw + 1],
                                                flat, AX.X, ALU.min)
                        nc.vector.tensor_reduce(mm_sb[w][:, 1, bw:bw + 1],
                                                flat, AX.X, ALU.max)
                        if b % WB != WB - 1:
                            continue
                        minmax_chain(w)
                        # normalize + write out this completed wave
                        for bb in range(w * WB, (w + 1) * WB):
                            bw2 = bb % WB
                            ot = outp.tile([IC, 2, 3, IMG], F32, tag="out")
                            src = img_sb[bb][:]
                            for ch in range(3):
                                dst = ot[:, :, ch, :].rearrange(
                                    "p a (g j) -> p a g j", g=2)
                                if (bb * 3 + ch) % 3 == 0:
                                    nc.scalar.activation(
                                        dst, src, AF.Identity,
                                        bias=scb[w][:, 1, ch, bw2:bw2 + 1],
                                        scale=scb[w][:, 0, ch, bw2:bw2 + 1])
                                else:
                                    nc.vector.tensor_scalar(
                                        dst, src,
                                        scb[w][:, 0, ch, bw2:bw2 + 1],
                                        scb[w][:, 1, ch, bw2:bw2 + 1],
                                        ALU.mult, ALU.add)
                            for ic in range(2):
                                nc.sync.dma_start(
                                    out_d[bb, :, ic * IC:(ic + 1) * IC, :]
                                    .rearrange("c i j -> i c j"),
                                    ot[:, ic, :, :])
    if split_waits:
        _split_multi_waits(nc)
    return nc


def _split_multi_waits(nc):
    """walrus on this toolchain accepts at most one sync wait per
    instruction; hoist extra waits onto same-engine NoOps placed before."""
    import bass_rust

    n_split = 0
    for fn in nc.m.functions:
        for bb in fn.blocks:
            out = []
            for ins in bb.instructions:
                si = ins.sync_info
                if si is not None and len(si.on_wait) > 1:
                    waits = list(si.on_wait)
                    for j, w in enumerate(waits[:-1]):
                        nop = bass_rust.InstNoOp(name=f"{ins.name}-sw{j}")
                        nop.engine = ins.engine
                        nop.sync_info = bass_rust.SyncInfo(
                            on_wait=[w], on_update=[])
                        out.append(nop)
                        n_split += 1
                    ins.sync_info = bass_rust.SyncInfo(
                        on_wait=[waits[-1]], on_update=list(si.on_update))
                out.append(ins)
            bb.instructions = out
    return n_split


def _get_nc():
    if "nc" not in _NC_CACHE:
        _NC_CACHE["nc"] = _build_nc()
    return _NC_CACHE["nc"]


def kernel(x, mean, std):
    x = np.asarray(x)
    if x.ndim == 3:
        x = x[:, 0, :]
    x = np.ascontiguousarray(x, dtype=np.float32)
    mean = np.ascontiguousarray(np.asarray(mean, np.float32).reshape(1, 3, 1, 1))
    std = np.ascontiguousarray(np.asarray(std, np.float32).reshape(1, 3, 1, 1))
    assert x.shape == (B_FULL, N), x.shape

    from concourse.bass_utils import run_bass_kernel_spmd

    nc = _get_nc()
    in_maps = [
        {"x": np.ascontiguousarray(x[i * B_LOC:(i + 1) * B_LOC]),
         "mean": mean, "std": std}
        for i in range(N_CORES)
    ]
    res = run_bass_kernel_spmd(nc, in_maps, list(range(N_CORES)))
    return np.concatenate(
        [res.results[i]["out"] for i in range(N_CORES)], axis=0)


if __name__ == "__main__":
    consts = build_consts()
    print("TOTD =", consts["totd"])
    print("tmask nonzero per jc:",
          [sum(1 for t in range(NT) if consts["tmask"][t][jc])
           for jc in range(2)])


# revision 65
# speedup vs baseline: 1.0221x; 1.0221x over previous
"""CWT scalogram layer on Trainium2 (Bass/Tile), 8-core data-parallel.

Pipeline per core (16 batch rows):
  1. fold x into X[q, col] layout (q = position mod 128) via PE transposes
  2. CWT: per scale, banded-Toeplitz matmuls accumulate in PSUM; Abs-evict
     to a bf16 scalogram tile ScAll[q, scale, col]
  3. time resize 2048->224 as accumulating matmuls over time-blocks
  4. PE transpose to put scales on partitions; scale resize 64->224
  5. per-image min/max, fused (img - mn)/(mx - mn + eps) then channel
     mean/std affine during PSUM->SBUF eviction; DMA out
"""

import sys
import numpy as np

for _p in ("/opt/trn_rl_repo",):
    if _p not in sys.path:
        sys.path.insert(0, _p)

import ml_dtypes

# ---------------------------------------------------------------------------
# problem constants (hardcoded per spec)
# ---------------------------------------------------------------------------
B_FULL = 128
N_CORES = 8
B_LOC = B_FULL // N_CORES          # 16
N = 2048
SCALES = 64
IMG = 224
PRECISION = 10
P = 128
NT = N // P                        # 16 time blocks
GAP = 5                            # zero blocks between batches (>= max |d|)
STRIDE = NT + GAP                  # 21 columns per batch in packed layout
COLS = B_LOC * STRIDE + GAP        # 341 packed columns (incl. tail gap)
XW = COLS + 2 * GAP + 1            # 352, X buffer with +-5 margin (even size)
IC = IMG // 2                      # 112, i/j chunk


# ---------------------------------------------------------------------------
# numpy constant builders
# ---------------------------------------------------------------------------
def _int_psi_morlet():
    M = 2 ** PRECISION
    xs = np.linspace(-8.0, 8.0, M)
    psi = np.exp(-xs ** 2 / 2.0) * np.cos(5.0 * xs)
    step = xs[1] - xs[0]
    return np.cumsum(psi) * step, step


def _scale_kernels():
    int_psi, step = _int_psi_morlet()
    kernels = []
    for s in range(1, SCALES + 1):
        j = np.floor(np.arange(16 * s + 1) / (s * step)).astype(np.int64)
        j = j[j < int_psi.size]
        kernels.append(int_psi[j].astype(np.float32))
    return kernels


def _wfull():
    """W[s-1, delta+513] so that coef[b,s,n] = sum_d W[s-1,d+513] x[b,n+d]."""
    kernels = _scale_kernels()
    W = np.zeros((SCALES, 1026), dtype=np.float64)
    for s in range(1, SCALES + 1):
        k = kernels[s - 1].astype(np.float64)
        dk = np.concatenate([[-k[0]], k[:-1] - k[1:], [k[-1]]])  # len 16s+2
        deltas = np.arange(-8 * s - 1, 8 * s + 1)
        W[s - 1, deltas + 513] = -np.sqrt(float(s)) * dk
    return W


def _d_range(s):
    d_min = -((8 * s + P) // P)
    d_max = (8 * s + P - 1) // P
    return list(range(d_min, d_max + 1))


def _build_cwt_weights():
    """Stationary matrices A_{s,d}[q, m] = W[s, q - m + 128 d + 513]."""
    W = _wfull()
    q = np.arange(P)[:, None]
    m = np.arange(P)[None, :]
    mats = []          # list of (s, d, [128,128] float32)
    d_lists = []
    for s in range(1, SCALES + 1):
        ds_ = _d_range(s)
        d_lists.append(ds_)
        row = W[s - 1]
        for d in ds_:
            idx = q - m + P * d + 513
            ok = (idx >= 0) & (idx <= 1025)
            a = np.where(ok, row[np.clip(idx, 0, 1025)], 0.0)
            mats.append(a.astype(np.float32))
    return np.stack(mats), d_lists   # [TOTD, 128, 128]


def _keys_cubic(x):
    out = ((1.5 * x - 2.5) * x) * x + 1.0
    out = np.where(x >= 1.0, ((-0.5 * x + 2.5) * x - 4.0) * x + 2.0, out)
    return np.where(x >= 2.0, 0.0, out)


def _resize_wmat(n_in, n_out):
    """Replica of jax.image.resize(method='cubic') weights: [n_in, n_out]."""
    scale = np.float32(n_out) / np.float32(n_in)
    inv_scale = np.float32(1.0) / scale
    kernel_scale = max(inv_scale, np.float32(1.0))
    sample_f = ((np.arange(n_out, dtype=np.float32) + 0.5) * inv_scale
                - np.float32(0.5))
    x = np.abs(sample_f[None, :]
               - np.arange(n_in, dtype=np.float32)[:, None]) / kernel_scale
    w = _keys_cubic(x.astype(np.float32)).astype(np.float32)
    total = w.sum(axis=0, keepdims=True)
    w = np.where(np.abs(total) > 1000.0 * np.finfo(np.float32).eps,
                 w / np.where(total != 0, total, 1), 0.0)
    ok = (sample_f >= -0.5) & (sample_f <= n_in - 0.5)
    return np.where(ok[None, :], w, 0.0).astype(np.float32)


def build_consts():
    A, d_lists = _build_cwt_weights()          # [TOTD,128,128]
    totd = A.shape[0]
    # DRAM layout [q, jidx, m] so each per-scale DMA reads contiguous chunks
    W_ALL = np.ascontiguousarray(A.transpose(1, 0, 2)).reshape(P, totd * P)
    wt = _resize_wmat(N, IMG)                  # [2048, 224]
    MT = np.ascontiguousarray(
        wt.reshape(NT, P, IMG).transpose(1, 0, 2)).reshape(P, NT * IMG)
    tmask = [[bool(np.any(wt[t * P:(t + 1) * P, jc * IC:(jc + 1) * IC]))
              for jc in range(2)] for t in range(NT)]
    MS = _resize_wmat(SCALES, IMG)             # [64, 224]
    eye112 = np.eye(IC, dtype=np.float32)
    bf16 = ml_dtypes.bfloat16
    return dict(
        W_ALL=W_ALL.astype(bf16),
        MT=MT.astype(bf16),
        MS=MS.astype(bf16),
        eye112=eye112,
        d_lists=d_lists,
        tmask=tmask,
        totd=totd,
    )


# ---------------------------------------------------------------------------
# golden numpy model of the device computation (for offline validation)
# ---------------------------------------------------------------------------
def golden(x, mean, std, quant=True):
    """Emulate the device computation in numpy. x: (B_LOC, N) float32."""
    c = build_consts()
    bf16 = ml_dtypes.bfloat16

    def q16(a):
        return a.astype(bf16).astype(np.float32) if quant else a.astype(np.float32)

    A, d_lists = _build_cwt_weights()
    xq = q16(x)
    # X buffer [128, XW]
    X = np.zeros((P, XW), dtype=np.float32)
    for b in range(B_LOC):
        for t in range(NT):
            X[:, GAP + STRIDE * b + t] = xq[b, t * P:(t + 1) * P]
    X = q16(X)
    # CWT
    scall = np.zeros((P, SCALES, COLS), dtype=np.float32)
    j = 0
    for si in range(SCALES):
        ps = np.zeros((P, COLS), dtype=np.float32)
        for d in d_lists[si]:
            a = q16(A[j]); j += 1
            ps += a.T @ X[:, GAP + d: GAP + d + COLS]
        scall[:, si, :] = np.abs(ps)
    scall = q16(scall)
    # time resize
    wt = q16(_resize_wmat(N, IMG))
    img1 = np.zeros((IMG, B_LOC, SCALES), dtype=np.float32)
    for t in range(NT):
        blk = wt[t * P:(t + 1) * P, :]             # [128, 224]
        sc = scall[:, :, [STRIDE * b + t for b in range(B_LOC)]]  # [128,S,B]
        img1 += np.einsum('qj,qsb->jbs', blk, sc)
    img1 = q16(img1)
    # scale resize
    ws = q16(_resize_wmat(SCALES, IMG))
    img = q16(np.einsum('si,jbs->bij', ws, img1))  # [B, 224i, 224j]
    mn = img.min(axis=(1, 2), keepdims=True)
    mx = img.max(axis=(1, 2), keepdims=True)
    r = 1.0 / (mx - mn + 1e-8)
    imgn = (img - mn) * r
    mean = np.asarray(mean, np.float32).reshape(3)
    std = np.asarray(std, np.float32).reshape(3)
    out = (imgn[:, None] - mean[None, :, None, None]) / std[None, :, None, None]
    return out.astype(np.float32)


# ---------------------------------------------------------------------------
# bass kernel
# ---------------------------------------------------------------------------
_NC_CACHE = {}


def _build_nc(split_waits=True):
    import concourse.bass as bass
    import concourse.mybir as mybir
    import concourse.tile as tile
    from concourse.tile_rust import add_dep_helper

    c = build_consts()
    d_lists = c["d_lists"]
    tmask = c["tmask"]
    F32 = mybir.dt.float32
    BF16 = mybir.dt.bfloat16
    AF = mybir.ActivationFunctionType
    ALU = mybir.AluOpType
    AX = mybir.AxisListType

    nc = bass.Bass()

    x_in = nc.dram_tensor("x", [B_LOC, N], F32, kind="ExternalInput")
    mean_in = nc.dram_tensor("mean", [1, 3, 1, 1], F32, kind="ExternalInput")
    std_in = nc.dram_tensor("std", [1, 3, 1, 1], F32, kind="ExternalInput")
    out_d = nc.dram_tensor("out", [B_LOC, 3, IMG, IMG], F32,
                           kind="ExternalOutput")

    w_all_d = nc.inline_tensor(np.asarray(c["W_ALL"]), name="w_all")
    mt_d = nc.inline_tensor(np.asarray(c["MT"]), name="mt_w")
    ms_d = nc.inline_tensor(np.asarray(c["MS"]), name="ms_w")
    eye_d = nc.inline_tensor(c["eye112"], name="eye112")

    with tile.TileContext(nc) as tc:
        with tc.tile_pool(name="const", bufs=1) as cpool:
            x_sb = cpool.tile([B_LOC, N], F32)
            nc.sync.dma_start(x_sb[:], x_in[:])
            eye = cpool.tile([IC, IC], F32)
            nc.sync.dma_start(eye[:], eye_d[:])
            mt_sb = cpool.tile([P, NT, IMG], BF16)
            nc.sync.dma_start(
                mt_sb[:], mt_d[:].rearrange("q (t j) -> q t j", j=IMG))
            # Ms weights duplicated in both partition halves so the
            # scale-resize lhsT base partition can match its rhs
            ms_sb = cpool.tile([P, IMG], BF16)
            nc.sync.dma_start(ms_sb[0:SCALES, :], ms_d[:])
            nc.sync.dma_start(ms_sb[SCALES:P, :], ms_d[:])
            ms_row = cpool.tile([1, 2, 3], F32)
            nc.sync.dma_start(ms_row[:, 0, :],
                              mean_in[:].rearrange("a b c d -> a (b c d)"))
            nc.sync.dma_start(ms_row[:, 1, :],
                              std_in[:].rearrange("a b c d -> a (b c d)"))
            ones_sb = cpool.tile([1, IC], F32)
            nc.vector.memset(ones_sb[:], 1.0)

            xbuf = cpool.tile([P, XW], BF16)
            nc.scalar.memzero(xbuf[:])
            # [q, t, s, b]: the time-resize moving operand (fixed t, all
            # (s, b)) is then fully contiguous, and the CWT eviction writes
            # 16-element contiguous runs (b innermost)
            scall = cpool.tile([P, NT, SCALES, B_LOC], BF16)
            img1 = cpool.tile([IC, 2, B_LOC, 2, SCALES // 2], F32)
            t_full = cpool.tile([P, 2, B_LOC // 2, IC], BF16)
            # min/max scalar chain state, duplicated per 8-batch wave so the
            # second wave's reductions overlap the first wave's normalize
            NW, WB = 2, B_LOC // 2
            mm_sb = [cpool.tile([IC, 2, 32], F32, name=f"mmsb{w}")
                     for w in range(NW)]
            mm_r = [cpool.tile([64, 1], F32, name=f"mmr{w}")
                    for w in range(NW)]
            row = [cpool.tile([1, 64], F32, name=f"row{w}") for w in range(NW)]
            sc_rng = cpool.tile([1, B_LOC], F32)
            sc_r = cpool.tile([1, B_LOC], F32)
            sc_b0 = cpool.tile([1, B_LOC], F32)
            inv_std = cpool.tile([1, 3], F32)
            ninv_std = cpool.tile([1, 3], F32)
            scrow = cpool.tile([1, NW, 2, 3, WB], F32)
            scb = [cpool.tile([IC, 2, 3, WB], F32, name=f"scb{w}")
                   for w in range(NW)]
            # unused padding slots flow through the PE transposes; zero them
            for w in range(NW):
                nc.vector.memset(mm_sb[w][:], 0.0)
                nc.vector.memset(mm_r[w][:], 0.0)

            # stage 1: fold x into xbuf via PE transposes -------------------
            # all 16 transposes target disjoint column ranges of ONE psum
            # bank (start=True only clears has_written bits, not data), so
            # a single merged eviction suffices.
            xb_v = xbuf[:, GAP:GAP + B_LOC * STRIDE].rearrange(
                "q (b u) -> q b u", u=STRIDE)
            with tc.tile_pool(name="pfold", bufs=1, space="PSUM") as pfold:
                pt = pfold.tile([P, NT, B_LOC], F32)
                for t in range(NT):
                    nc.tensor.transpose(
                        pt[:, t, :], x_sb[:, t * P:(t + 1) * P],
                        eye[:B_LOC, :B_LOC])
                nc.scalar.activation(
                    xb_v[:, :, 0:NT].rearrange("q b u -> q u b"),
                    pt[:], AF.Copy)

            # stage 2+3: CWT scales + time-resize interleaved ---------------
            # weight DMAs grouped several scales at a time: bigger transfers
            # and one DMA semaphore per group (first group small so the PE
            # can start early)
            GRPS = [list(range(*r)) for r in
                    [(0, 2), (2, 8), (8, 16), (16, 24), (24, 32),
                     (32, 40), (40, 48), (48, 56), (56, 64)]]
            n_grp = len(GRPS)
            grp_nd = [sum(len(d_lists[si]) for si in g) for g in GRPS]
            nd_max = max(grp_nd)

            with (
                tc.tile_pool(name="pcwt", bufs=3, space="PSUM") as pcwt,
                tc.tile_pool(name="wpool", bufs=2) as wpool,
                tc.tile_pool(name="ptres", bufs=1, space="PSUM") as ptres,
            ):
                w_v = w_all_d[:].rearrange("q (j m) -> q j m", m=P)
                grp_off = [sum(grp_nd[:g]) for g in range(n_grp)]

                def cwt_group(g):
                    wt_t = wpool.tile([P, nd_max, P], BF16, tag="wt")
                    nc.sync.dma_start(
                        wt_t[:, 0:grp_nd[g], :],
                        w_v[:, grp_off[g]:grp_off[g] + grp_nd[g], :])
                    k0 = 0
                    for si in GRPS[g]:
                        ds_ = d_lists[si]
                        ps = pcwt.tile([P, COLS], F32, tag="cwtps")
                        for k, d in enumerate(ds_):
                            nc.tensor.matmul(
                                ps[:], wt_t[:, k0 + k, :],
                                xbuf[:, GAP + d: GAP + d + COLS],
                                start=(k == 0), stop=(k == len(ds_) - 1))
                        k0 += len(ds_)
                        src = ps[:, 0:B_LOC * STRIDE].rearrange(
                            "p (b u) -> p u b", u=STRIDE)[:, 0:NT, :]
                        nc.scalar.activation(scall[:, :, si, :], src, AF.Abs)

                # time-resize per scale-half, interleaved into the CWT
                # stream; t outer with the two jc psum banks round-robined
                # so consecutive matmuls never accumulate into the same bank
                sc_flat = scall[:].rearrange("q t s b -> q (t s b)")

                def tres_half(h):
                    ps2 = [ptres.tile([IC, 512], F32, name=f"tres{h}{jc}")
                           for jc in range(2)]
                    kidx = [0, 0]
                    n_t = [sum(1 for tt in range(NT) if tmask[tt][jc])
                           for jc in range(2)]
                    for t in range(NT):
                        for jc in range(2):
                            if not tmask[t][jc]:
                                continue
                            off = (t * SCALES + h * 32) * B_LOC
                            nc.tensor.matmul(
                                ps2[jc],
                                mt_sb[:, t, jc * IC:(jc + 1) * IC],
                                sc_flat[:, off:off + 512],
                                start=(kidx[jc] == 0),
                                stop=(kidx[jc] == n_t[jc] - 1))
                            kidx[jc] += 1
                    for jc in range(2):
                        # psum cols are (s, b); reorder to (b, s) on the way
                        # out so the transpose blocks stay (b-pair, h, s)
                        nc.scalar.activation(
                            img1[:, jc, :, h, :],
                            ps2[jc][:].rearrange("p (s b) -> p b s",
                                                 b=B_LOC), AF.Copy)

                for g in range(5):
                    cwt_group(g)
                tres_half(0)
                for g in range(5, n_grp):
                    cwt_group(g)
                tres_half(1)

            # stage 4: transpose + scale-resize + minmax --------------------
            with (
                tc.tile_pool(name="ptp", bufs=3, space="PSUM") as ptp,
                tc.tile_pool(name="pimg", bufs=3, space="PSUM") as pimg,
                tc.tile_pool(name="pmisc", bufs=2, space="PSUM") as pmisc,
                tc.tile_pool(name="imgsb", bufs=B_LOC) as imgsb_pool,
            ):
                for jc in range(2):
                    for k in range(B_LOC // 2):
                        pt = ptp.tile([P, IC], F32, tag="tp")
                        nc.tensor.transpose(
                            pt[:], img1[:, jc, 2 * k:2 * k + 2, :, :], eye[:])
                        nc.scalar.activation(t_full[:, jc, k, :], pt[:],
                                             AF.Copy)

                def minmax_chain(w):
                    """min/max partials of wave w -> per-image affine scb."""
                    pt1 = ptp.tile([P, IC], F32, tag="tp", name=f"pt1{w}")
                    nc.tensor.transpose(
                        pt1[0:64, :],
                        mm_sb[w][:].rearrange("p a b -> p (a b)"), eye[:])
                    nc.vector.tensor_reduce(mm_r[w][0:WB, :], pt1[0:WB, :],
                                            AX.X, ALU.min)
                    nc.vector.tensor_reduce(mm_r[w][32:32 + WB, :],
                                            pt1[32:32 + WB, :],
                                            AX.X, ALU.max)
                    pt2 = pmisc.tile([P, IC], F32, tag="misc", name=f"pt2{w}")
                    nc.tensor.transpose(pt2[0:1, 0:64], mm_r[w][:],
                                        eye[0:64, 0:64])
                    nc.vector.tensor_copy(row[w][:], pt2[0:1, 0:64])

                    rng = sc_rng[:, w * WB:(w + 1) * WB]
                    r_ = sc_r[:, w * WB:(w + 1) * WB]
                    b0 = sc_b0[:, w * WB:(w + 1) * WB]
                    nc.vector.tensor_tensor(rng, row[w][:, 32:32 + WB],
                                            row[w][:, 0:WB], ALU.subtract)
                    nc.vector.tensor_scalar_add(rng, rng, 1e-8)
                    nc.vector.reciprocal(r_, rng)
                    nc.vector.tensor_tensor(b0, row[w][:, 0:WB], r_,
                                            ALU.mult)
                    if w == 0:
                        nc.vector.reciprocal(inv_std[:], ms_row[:, 1, :])
                        nc.vector.tensor_scalar_mul(ninv_std[:], inv_std[:],
                                                    -1.0)
                    for ch in range(3):
                        nc.vector.tensor_scalar(
                            scrow[:, w, 0, ch, :], r_,
                            inv_std[:, ch:ch + 1], None, ALU.mult)
                        nc.vector.tensor_scalar(
                            scrow[:, w, 1, ch, :], b0,
                            ms_row[:, 0, ch:ch + 1], ninv_std[:, ch:ch + 1],
                            ALU.add, ALU.mult)
                    pbc = pmisc.tile([P, IC], F32, tag="misc", name=f"pbc{w}")
                    nc.tensor.matmul(
                        pbc[0:IC, 0:6 * WB], ones_sb[:],
                        scrow[:, w].rearrange("p a c b -> p (a c b)"),
                        start=True, stop=True)
                    nc.scalar.activation(
                        scb[w][:].rearrange("p a c b -> p (a c b)"),
                        pbc[0:IC, 0:6 * WB], AF.Copy)

                img_sb = []
                with tc.tile_pool(name="outp", bufs=4) as outp:
                    for b in range(B_LOC):
                        k, bl = b // 2, b % 2
                        tsrc = t_full[64 * bl:64 * (bl + 1)]
                        ms_half = ms_sb[64 * bl:64 * (bl + 1)]
                        isb = imgsb_pool.tile([IC, 2, 2, IC], BF16,
                                              tag="imgsb")
                        img_sb.append(isb)
                        # all 4 (ic, jc) chunks go into ONE psum bank;
                        # single merged eviction + one min/max pass each
                        pi = pimg.tile([IC, 2, 2, IC], F32, tag="img")
                        for ic in range(2):
                            for jc in range(2):
                                nc.tensor.matmul(
                                    pi[:, ic, jc, :],
                                    ms_half[:, ic * IC:(ic + 1) * IC],
                                    tsrc[:, jc, k, :], start=True, stop=True)
                        nc.scalar.activation(isb[:], pi[:], AF.Copy)
                        w, bw = b // WB, b % WB
                        flat = isb[:].rearrange("p a b j -> p (a b j)")
                        nc.vector.tensor_reduce(mm_sb[w][:, 0, bw:bw + 1],
                                                flat, AX.X, ALU.min)
                        nc.vector.tensor_reduce(mm_sb[w][:, 1, bw:bw + 1],
                                                flat, AX.X, ALU.max)
                        if b % WB != WB - 1:
                            continue
                        minmax_chain(w)
                        # normalize + write out this completed wave
                        for bb in range(w * WB, (w + 1) * WB):
                            bw2 = bb % WB
                            ot = outp.tile([IC, 2, 3, IMG], F32, tag="out")
                            src = img_sb[bb][:]
                            for ch in range(3):
                                dst = ot[:, :, ch, :].rearrange(
                                    "p a (g j) -> p a g j", g=2)
                                if (bb * 3 + ch) % 3 == 0:
                                    nc.scalar.activation(
                                        dst, src, AF.Identity,
                                        bias=scb[w][:, 1, ch, bw2:bw2 + 1],
                                        scale=scb[w][:, 0, ch, bw2:bw2 + 1])
                                else:
                                    nc.vector.tensor_scalar(
                                        dst, src,
                                        scb[w][:, 0, ch, bw2:bw2 + 1],
                                        scb[w][:, 1, ch, bw2:bw2 + 1],
                                        ALU.mult, ALU.add)
                            for ic in range(2):
                                nc.sync.dma_start(
                                    out_d[bb, :, ic * IC:(ic + 1) * IC, :]
                                    .rearrange("c i j -> i c j"),
                                    ot[:, ic, :, :])
    if split_waits:
        _split_multi_waits(nc)
    return nc


def _split_multi_waits(nc):
    """walrus on this toolchain accepts at most one sync wait per
    instruction; hoist extra waits onto same-engine NoOps placed before."""
    import bass_rust

    n_split = 0
    for fn in nc.m.functions:
        for bb in fn.blocks:
            out = []
            for ins in bb.instructions:
                si = ins.sync_info
                if si is not None and len(si.on_wait) > 1:
                    waits = list(si.on_wait)
                    for j, w in enumerate(waits[:-1]):
                        nop = bass_rust.InstNoOp(name=f"{ins.name}-sw{j}")
                        nop.engine = ins.engine
                        nop.sync_info = bass_rust.SyncInfo(
                            on_wait=[w], on_update=[])
                        out.append(nop)
                        n_split += 1
                    ins.sync_info = bass_rust.SyncInfo(
                        on_wait=[waits[-1]], on_update=list(si.on_update))
                out.append(ins)
            bb.instructions = out
    return n_split


def _get_nc():
    if "nc" not in _NC_CACHE:
        _NC_CACHE["nc"] = _build_nc()
    return _NC_CACHE["nc"]


def kernel(x, mean, std):
    x = np.asarray(x)
    if x.ndim == 3:
        x = x[:, 0, :]
    x = np.ascontiguousarray(x, dtype=np.float32)
    mean = np.ascontiguousarray(np.asarray(mean, np.float32).reshape(1, 3, 1, 1))
    std = np.ascontiguousarray(np.asarray(std, np.float32).reshape(1, 3, 1, 1))
    assert x.shape == (B_FULL, N), x.shape

    from concourse.bass_utils import run_bass_kernel_spmd

    nc = _get_nc()
    in_maps = [
        {"x": np.ascontiguousarray(x[i * B_LOC:(i + 1) * B_LOC]),
         "mean": mean, "std": std}
        for i in range(N_CORES)
    ]
    res = run_bass_kernel_spmd(nc, in_maps, list(range(N_CORES)))
    return np.concatenate(
        [res.results[i]["out"] for i in range(N_CORES)], axis=0)


if __name__ == "__main__":
    consts = build_consts()
    print("TOTD =", consts["totd"])
    print("tmask nonzero per jc:",
          [sum(1 for t in range(NT) if consts["tmask"][t][jc])
           for jc in range(2)])


# revision 66
# speedup vs baseline: 1.0476x; 1.0249x over previous
"""CWT scalogram layer on Trainium2 (Bass/Tile), 8-core data-parallel.

Pipeline per core (16 batch rows):
  1. fold x into X[q, col] layout (q = position mod 128) via PE transposes
  2. CWT: per scale, banded-Toeplitz matmuls accumulate in PSUM; Abs-evict
     to a bf16 scalogram tile ScAll[q, scale, col]
  3. time resize 2048->224 as accumulating matmuls over time-blocks
  4. PE transpose to put scales on partitions; scale resize 64->224
  5. per-image min/max, fused (img - mn)/(mx - mn + eps) then channel
     mean/std affine during PSUM->SBUF eviction; DMA out
"""

import sys
import numpy as np

for _p in ("/opt/trn_rl_repo",):
    if _p not in sys.path:
        sys.path.insert(0, _p)

import ml_dtypes

# ---------------------------------------------------------------------------
# problem constants (hardcoded per spec)
# ---------------------------------------------------------------------------
B_FULL = 128
N_CORES = 8
B_LOC = B_FULL // N_CORES          # 16
N = 2048
SCALES = 64
IMG = 224
PRECISION = 10
P = 128
NT = N // P                        # 16 time blocks
GAP = 5                            # zero blocks between batches (>= max |d|)
STRIDE = NT + GAP                  # 21 columns per batch in packed layout
COLS = B_LOC * STRIDE + GAP        # 341 packed columns (incl. tail gap)
XW = COLS + 2 * GAP + 1            # 352, X buffer with +-5 margin (even size)
IC = IMG // 2                      # 112, i/j chunk


# ---------------------------------------------------------------------------
# numpy constant builders
# ---------------------------------------------------------------------------
def _int_psi_morlet():
    M = 2 ** PRECISION
    xs = np.linspace(-8.0, 8.0, M)
    psi = np.exp(-xs ** 2 / 2.0) * np.cos(5.0 * xs)
    step = xs[1] - xs[0]
    return np.cumsum(psi) * step, step


def _scale_kernels():
    int_psi, step = _int_psi_morlet()
    kernels = []
    for s in range(1, SCALES + 1):
        j = np.floor(np.arange(16 * s + 1) / (s * step)).astype(np.int64)
        j = j[j < int_psi.size]
        kernels.append(int_psi[j].astype(np.float32))
    return kernels


def _wfull():
    """W[s-1, delta+513] so that coef[b,s,n] = sum_d W[s-1,d+513] x[b,n+d]."""
    kernels = _scale_kernels()
    W = np.zeros((SCALES, 1026), dtype=np.float64)
    for s in range(1, SCALES + 1):
        k = kernels[s - 1].astype(np.float64)
        dk = np.concatenate([[-k[0]], k[:-1] - k[1:], [k[-1]]])  # len 16s+2
        deltas = np.arange(-8 * s - 1, 8 * s + 1)
        W[s - 1, deltas + 513] = -np.sqrt(float(s)) * dk
    return W


def _d_range(s):
    d_min = -((8 * s + P) // P)
    d_max = (8 * s + P - 1) // P
    return list(range(d_min, d_max + 1))


def _build_cwt_weights():
    """Stationary matrices A_{s,d}[q, m] = W[s, q - m + 128 d + 513]."""
    W = _wfull()
    q = np.arange(P)[:, None]
    m = np.arange(P)[None, :]
    mats = []          # list of (s, d, [128,128] float32)
    d_lists = []
    for s in range(1, SCALES + 1):
        ds_ = _d_range(s)
        d_lists.append(ds_)
        row = W[s - 1]
        for d in ds_:
            idx = q - m + P * d + 513
            ok = (idx >= 0) & (idx <= 1025)
            a = np.where(ok, row[np.clip(idx, 0, 1025)], 0.0)
            mats.append(a.astype(np.float32))
    return np.stack(mats), d_lists   # [TOTD, 128, 128]


def _keys_cubic(x):
    out = ((1.5 * x - 2.5) * x) * x + 1.0
    out = np.where(x >= 1.0, ((-0.5 * x + 2.5) * x - 4.0) * x + 2.0, out)
    return np.where(x >= 2.0, 0.0, out)


def _resize_wmat(n_in, n_out):
    """Replica of jax.image.resize(method='cubic') weights: [n_in, n_out]."""
    scale = np.float32(n_out) / np.float32(n_in)
    inv_scale = np.float32(1.0) / scale
    kernel_scale = max(inv_scale, np.float32(1.0))
    sample_f = ((np.arange(n_out, dtype=np.float32) + 0.5) * inv_scale
                - np.float32(0.5))
    x = np.abs(sample_f[None, :]
               - np.arange(n_in, dtype=np.float32)[:, None]) / kernel_scale
    w = _keys_cubic(x.astype(np.float32)).astype(np.float32)
    total = w.sum(axis=0, keepdims=True)
    w = np.where(np.abs(total) > 1000.0 * np.finfo(np.float32).eps,
                 w / np.where(total != 0, total, 1), 0.0)
    ok = (sample_f >= -0.5) & (sample_f <= n_in - 0.5)
    return np.where(ok[None, :], w, 0.0).astype(np.float32)


def build_consts():
    A, d_lists = _build_cwt_weights()          # [TOTD,128,128]
    totd = A.shape[0]
    # DRAM layout [q, jidx, m] so each per-scale DMA reads contiguous chunks
    W_ALL = np.ascontiguousarray(A.transpose(1, 0, 2)).reshape(P, totd * P)
    wt = _resize_wmat(N, IMG)                  # [2048, 224]
    MT = np.ascontiguousarray(
        wt.reshape(NT, P, IMG).transpose(1, 0, 2)).reshape(P, NT * IMG)
    tmask = [[bool(np.any(wt[t * P:(t + 1) * P, jc * IC:(jc + 1) * IC]))
              for jc in range(2)] for t in range(NT)]
    MS = _resize_wmat(SCALES, IMG)             # [64, 224]
    eye112 = np.eye(IC, dtype=np.float32)
    bf16 = ml_dtypes.bfloat16
    return dict(
        W_ALL=W_ALL.astype(bf16),
        MT=MT.astype(bf16),
        MS=MS.astype(bf16),
        eye112=eye112,
        d_lists=d_lists,
        tmask=tmask,
        totd=totd,
    )


# ---------------------------------------------------------------------------
# golden numpy model of the device computation (for offline validation)
# ---------------------------------------------------------------------------
def golden(x, mean, std, quant=True):
    """Emulate the device computation in numpy. x: (B_LOC, N) float32."""
    c = build_consts()
    bf16 = ml_dtypes.bfloat16

    def q16(a):
        return a.astype(bf16).astype(np.float32) if quant else a.astype(np.float32)

    A, d_lists = _build_cwt_weights()
    xq = q16(x)
    # X buffer [128, XW]
    X = np.zeros((P, XW), dtype=np.float32)
    for b in range(B_LOC):
        for t in range(NT):
            X[:, GAP + STRIDE * b + t] = xq[b, t * P:(t + 1) * P]
    X = q16(X)
    # CWT
    scall = np.zeros((P, SCALES, COLS), dtype=np.float32)
    j = 0
    for si in range(SCALES):
        ps = np.zeros((P, COLS), dtype=np.float32)
        for d in d_lists[si]:
            a = q16(A[j]); j += 1
            ps += a.T @ X[:, GAP + d: GAP + d + COLS]
        scall[:, si, :] = np.abs(ps)
    scall = q16(scall)
    # time resize
    wt = q16(_resize_wmat(N, IMG))
    img1 = np.zeros((IMG, B_LOC, SCALES), dtype=np.float32)
    for t in range(NT):
        blk = wt[t * P:(t + 1) * P, :]             # [128, 224]
        sc = scall[:, :, [STRIDE * b + t for b in range(B_LOC)]]  # [128,S,B]
        img1 += np.einsum('qj,qsb->jbs', blk, sc)
    img1 = q16(img1)
    # scale resize
    ws = q16(_resize_wmat(SCALES, IMG))
    img = q16(np.einsum('si,jbs->bij', ws, img1))  # [B, 224i, 224j]
    mn = img.min(axis=(1, 2), keepdims=True)
    mx = img.max(axis=(1, 2), keepdims=True)
    r = 1.0 / (mx - mn + 1e-8)
    imgn = (img - mn) * r
    mean = np.asarray(mean, np.float32).reshape(3)
    std = np.asarray(std, np.float32).reshape(3)
    out = (imgn[:, None] - mean[None, :, None, None]) / std[None, :, None, None]
    return out.astype(np.float32)


# ---------------------------------------------------------------------------
# bass kernel
# ---------------------------------------------------------------------------
_NC_CACHE = {}


def _build_nc(split_waits=True):
    import concourse.bass as bass
    import concourse.mybir as mybir
    import concourse.tile as tile
    from concourse.tile_rust import add_dep_helper

    c = build_consts()
    d_lists = c["d_lists"]
    tmask = c["tmask"]
    F32 = mybir.dt.float32
    BF16 = mybir.dt.bfloat16
    AF = mybir.ActivationFunctionType
    ALU = mybir.AluOpType
    AX = mybir.AxisListType

    nc = bass.Bass()

    x_in = nc.dram_tensor("x", [B_LOC, N], F32, kind="ExternalInput")
    mean_in = nc.dram_tensor("mean", [1, 3, 1, 1], F32, kind="ExternalInput")
    std_in = nc.dram_tensor("std", [1, 3, 1, 1], F32, kind="ExternalInput")
    out_d = nc.dram_tensor("out", [B_LOC, 3, IMG, IMG], F32,
                           kind="ExternalOutput")

    w_all_d = nc.inline_tensor(np.asarray(c["W_ALL"]), name="w_all")
    mt_d = nc.inline_tensor(np.asarray(c["MT"]), name="mt_w")
    ms_d = nc.inline_tensor(np.asarray(c["MS"]), name="ms_w")
    eye_d = nc.inline_tensor(c["eye112"], name="eye112")

    with tile.TileContext(nc) as tc:
        with tc.tile_pool(name="const", bufs=1) as cpool:
            x_sb = cpool.tile([B_LOC, N], F32)
            nc.sync.dma_start(x_sb[:], x_in[:])
            eye = cpool.tile([IC, IC], F32)
            nc.sync.dma_start(eye[:], eye_d[:])
            mt_sb = cpool.tile([P, NT, IMG], BF16)
            nc.sync.dma_start(
                mt_sb[:], mt_d[:].rearrange("q (t j) -> q t j", j=IMG))
            # Ms weights duplicated in both partition halves so the
            # scale-resize lhsT base partition can match its rhs
            ms_sb = cpool.tile([P, IMG], BF16)
            nc.sync.dma_start(ms_sb[0:SCALES, :], ms_d[:])
            nc.sync.dma_start(ms_sb[SCALES:P, :], ms_d[:])
            ms_row = cpool.tile([1, 2, 3], F32)
            nc.sync.dma_start(ms_row[:, 0, :],
                              mean_in[:].rearrange("a b c d -> a (b c d)"))
            nc.sync.dma_start(ms_row[:, 1, :],
                              std_in[:].rearrange("a b c d -> a (b c d)"))
            ones_sb = cpool.tile([1, IC], F32)
            nc.vector.memset(ones_sb[:], 1.0)

            xbuf = cpool.tile([P, XW], BF16)
            nc.scalar.memzero(xbuf[:])
            # [q, t, s, b]: the time-resize moving operand (fixed t, all
            # (s, b)) is then fully contiguous, and the CWT eviction writes
            # 16-element contiguous runs (b innermost)
            scall = cpool.tile([P, NT, SCALES, B_LOC], BF16)
            img1 = cpool.tile([IC, 2, B_LOC, 2, SCALES // 2], F32)
            t_full = cpool.tile([P, 2, B_LOC // 2, IC], BF16)
            # min/max scalar chain state, duplicated per 8-batch wave so the
            # second wave's reductions overlap the first wave's normalize
            NW, WB = 2, B_LOC // 2
            mm_sb = [cpool.tile([IC, 2, 32], F32, name=f"mmsb{w}")
                     for w in range(NW)]
            mm_r = [cpool.tile([64, 1], F32, name=f"mmr{w}")
                    for w in range(NW)]
            row = [cpool.tile([1, 64], F32, name=f"row{w}") for w in range(NW)]
            sc_rng = cpool.tile([1, B_LOC], F32)
            sc_r = cpool.tile([1, B_LOC], F32)
            sc_b0 = cpool.tile([1, B_LOC], F32)
            inv_std = cpool.tile([1, 3], F32)
            ninv_std = cpool.tile([1, 3], F32)
            scrow = cpool.tile([1, NW, 2, 3, WB], F32)
            scb = [cpool.tile([IC, 2, 3, WB], F32, name=f"scb{w}")
                   for w in range(NW)]
            # unused padding slots flow through the PE transposes; zero them
            for w in range(NW):
                nc.vector.memset(mm_sb[w][:], 0.0)
                nc.vector.memset(mm_r[w][:], 0.0)

            # stage 1: fold x into xbuf via PE transposes -------------------
            # all 16 transposes target disjoint column ranges of ONE psum
            # bank (start=True only clears has_written bits, not data), so
            # a single merged eviction suffices.
            xb_v = xbuf[:, GAP:GAP + B_LOC * STRIDE].rearrange(
                "q (b u) -> q b u", u=STRIDE)
            with tc.tile_pool(name="pfold", bufs=1, space="PSUM") as pfold:
                pt = pfold.tile([P, NT, B_LOC], F32)
                for t in range(NT):
                    nc.tensor.transpose(
                        pt[:, t, :], x_sb[:, t * P:(t + 1) * P],
                        eye[:B_LOC, :B_LOC])
                nc.scalar.activation(
                    xb_v[:, :, 0:NT].rearrange("q b u -> q u b"),
                    pt[:], AF.Copy)

            # stage 2+3: CWT scales + time-resize interleaved ---------------
            # weight DMAs grouped several scales at a time: bigger transfers
            # and one DMA semaphore per group (first group small so the PE
            # can start early)
            GRPS = [list(range(*r)) for r in
                    [(0, 2), (2, 8), (8, 16), (16, 24), (24, 32),
                     (32, 40), (40, 48), (48, 56), (56, 64)]]
            n_grp = len(GRPS)
            grp_nd = [sum(len(d_lists[si]) for si in g) for g in GRPS]
            nd_max = max(grp_nd)

            with (
                tc.tile_pool(name="pcwt", bufs=3, space="PSUM") as pcwt,
                tc.tile_pool(name="wpool", bufs=3) as wpool,
                tc.tile_pool(name="ptres", bufs=1, space="PSUM") as ptres,
            ):
                w_v = w_all_d[:].rearrange("q (j m) -> q j m", m=P)
                grp_off = [sum(grp_nd[:g]) for g in range(n_grp)]

                def cwt_group(g):
                    wt_t = wpool.tile([P, nd_max, P], BF16, tag="wt")
                    nc.sync.dma_start(
                        wt_t[:, 0:grp_nd[g], :],
                        w_v[:, grp_off[g]:grp_off[g] + grp_nd[g], :])
                    k0 = 0
                    for si in GRPS[g]:
                        ds_ = d_lists[si]
                        ps = pcwt.tile([P, COLS], F32, tag="cwtps")
                        for k, d in enumerate(ds_):
                            nc.tensor.matmul(
                                ps[:], wt_t[:, k0 + k, :],
                                xbuf[:, GAP + d: GAP + d + COLS],
                                start=(k == 0), stop=(k == len(ds_) - 1))
                        k0 += len(ds_)
                        src = ps[:, 0:B_LOC * STRIDE].rearrange(
                            "p (b u) -> p u b", u=STRIDE)[:, 0:NT, :]
                        nc.scalar.activation(scall[:, :, si, :], src, AF.Abs)

                # time-resize per scale-half, interleaved into the CWT
                # stream; t outer with the two jc psum banks round-robined
                # so consecutive matmuls never accumulate into the same bank
                sc_flat = scall[:].rearrange("q t s b -> q (t s b)")

                def tres_half(h):
                    ps2 = [ptres.tile([IC, 512], F32, name=f"tres{h}{jc}")
                           for jc in range(2)]
                    kidx = [0, 0]
                    n_t = [sum(1 for tt in range(NT) if tmask[tt][jc])
                           for jc in range(2)]
                    for t in range(NT):
                        for jc in range(2):
                            if not tmask[t][jc]:
                                continue
                            off = (t * SCALES + h * 32) * B_LOC
                            nc.tensor.matmul(
                                ps2[jc],
                                mt_sb[:, t, jc * IC:(jc + 1) * IC],
                                sc_flat[:, off:off + 512],
                                start=(kidx[jc] == 0),
                                stop=(kidx[jc] == n_t[jc] - 1))
                            kidx[jc] += 1
                    for jc in range(2):
                        # psum cols are (s, b); reorder to (b, s) on the way
                        # out so the transpose blocks stay (b-pair, h, s)
                        nc.scalar.activation(
                            img1[:, jc, :, h, :],
                            ps2[jc][:].rearrange("p (s b) -> p b s",
                                                 b=B_LOC), AF.Copy)

                for g in range(5):
                    cwt_group(g)
                tres_half(0)
                for g in range(5, n_grp):
                    cwt_group(g)
                tres_half(1)

            # stage 4: transpose + scale-resize + minmax --------------------
            with (
                tc.tile_pool(name="ptp", bufs=3, space="PSUM") as ptp,
                tc.tile_pool(name="pimg", bufs=3, space="PSUM") as pimg,
                tc.tile_pool(name="pmisc", bufs=2, space="PSUM") as pmisc,
                tc.tile_pool(name="imgsb", bufs=B_LOC) as imgsb_pool,
            ):
                for jc in range(2):
                    for k in range(B_LOC // 2):
                        pt = ptp.tile([P, IC], F32, tag="tp")
                        nc.tensor.transpose(
                            pt[:], img1[:, jc, 2 * k:2 * k + 2, :, :], eye[:])
                        nc.scalar.activation(t_full[:, jc, k, :], pt[:],
                                             AF.Copy)

                def minmax_chain(w):
                    """min/max partials of wave w -> per-image affine scb."""
                    pt1 = ptp.tile([P, IC], F32, tag="tp", name=f"pt1{w}")
                    nc.tensor.transpose(
                        pt1[0:64, :],
                        mm_sb[w][:].rearrange("p a b -> p (a b)"), eye[:])
                    nc.vector.tensor_reduce(mm_r[w][0:WB, :], pt1[0:WB, :],
                                            AX.X, ALU.min)
                    nc.vector.tensor_reduce(mm_r[w][32:32 + WB, :],
                                            pt1[32:32 + WB, :],
                                            AX.X, ALU.max)
                    pt2 = pmisc.tile([P, IC], F32, tag="misc", name=f"pt2{w}")
                    nc.tensor.transpose(pt2[0:1, 0:64], mm_r[w][:],
                                        eye[0:64, 0:64])
                    nc.vector.tensor_copy(row[w][:], pt2[0:1, 0:64])

                    rng = sc_rng[:, w * WB:(w + 1) * WB]
                    r_ = sc_r[:, w * WB:(w + 1) * WB]
                    b0 = sc_b0[:, w * WB:(w + 1) * WB]
                    nc.vector.tensor_tensor(rng, row[w][:, 32:32 + WB],
                                            row[w][:, 0:WB], ALU.subtract)
                    nc.vector.tensor_scalar_add(rng, rng, 1e-8)
                    nc.vector.reciprocal(r_, rng)
                    nc.vector.tensor_tensor(b0, row[w][:, 0:WB], r_,
                                            ALU.mult)
                    if w == 0:
                        nc.vector.reciprocal(inv_std[:], ms_row[:, 1, :])
                        nc.vector.tensor_scalar_mul(ninv_std[:], inv_std[:],
                                                    -1.0)
                    for ch in range(3):
                        nc.vector.tensor_scalar(
                            scrow[:, w, 0, ch, :], r_,
                            inv_std[:, ch:ch + 1], None, ALU.mult)
                        nc.vector.tensor_scalar(
                            scrow[:, w, 1, ch, :], b0,
                            ms_row[:, 0, ch:ch + 1], ninv_std[:, ch:ch + 1],
                            ALU.add, ALU.mult)
                    pbc = pmisc.tile([P, IC], F32, tag="misc", name=f"pbc{w}")
                    nc.tensor.matmul(
                        pbc[0:IC, 0:6 * WB], ones_sb[:],
                        scrow[:, w].rearrange("p a c b -> p (a c b)"),
                        start=True, stop=True)
                    nc.scalar.activation(
                        scb[w][:].rearrange("p a c b -> p (a c b)"),
                        pbc[0:IC, 0:6 * WB], AF.Copy)

                img_sb = []
                with tc.tile_pool(name="outp", bufs=8) as outp:
                    for b in range(B_LOC):
                        k, bl = b // 2, b % 2
                        tsrc = t_full[64 * bl:64 * (bl + 1)]
                        ms_half = ms_sb[64 * bl:64 * (bl + 1)]
                        isb = imgsb_pool.tile([IC, 2, 2, IC], BF16,
                                              tag="imgsb")
                        img_sb.append(isb)
                        # all 4 (ic, jc) chunks go into ONE psum bank;
                        # single merged eviction + one min/max pass each
                        pi = pimg.tile([IC, 2, 2, IC], F32, tag="img")
                        for ic in range(2):
                            for jc in range(2):
                                nc.tensor.matmul(
                                    pi[:, ic, jc, :],
                                    ms_half[:, ic * IC:(ic + 1) * IC],
                                    tsrc[:, jc, k, :], start=True, stop=True)
                        nc.scalar.activation(isb[:], pi[:], AF.Copy)
                        w, bw = b // WB, b % WB
                        flat = isb[:].rearrange("p a b j -> p (a b j)")
                        nc.vector.tensor_reduce(mm_sb[w][:, 0, bw:bw + 1],
                                                flat, AX.X, ALU.min)
                        nc.vector.tensor_reduce(mm_sb[w][:, 1, bw:bw + 1],
                                                flat, AX.X, ALU.max)
                        if b % WB != WB - 1:
                            continue
                        minmax_chain(w)
                        # normalize + write out this completed wave
                        for bb in range(w * WB, (w + 1) * WB):
                            bw2 = bb % WB
                            ot = outp.tile([IC, 2, 3, IMG], F32, tag="out")
                            src = img_sb[bb][:]
                            for ch in range(3):
                                dst = ot[:, :, ch, :].rearrange(
                                    "p a (g j) -> p a g j", g=2)
                                if (bb * 3 + ch) % 3 == 0:
                                    nc.scalar.activation(
                                        dst, src, AF.Identity,
                                        bias=scb[w][:, 1, ch, bw2:bw2 + 1],
                                        scale=scb[w][:, 0, ch, bw2:bw2 + 1])
                                else:
                                    nc.vector.tensor_scalar(
                                        dst, src,
                                        scb[w][:, 0, ch, bw2:bw2 + 1],
                                        scb[w][:, 1, ch, bw2:bw2 + 1],
                                        ALU.mult, ALU.add)
                            for ic in range(2):
                                nc.sync.dma_start(
                                    out_d[bb, :, ic * IC:(ic + 1) * IC, :]
                                    .rearrange("c i j -> i c j"),
                                    ot[:, ic, :, :])
    if split_waits:
        _split_multi_waits(nc)
    return nc


def _split_multi_waits(nc):
    """walrus on this toolchain accepts at most one sync wait per
    instruction; hoist extra waits onto same-engine NoOps placed before."""
    import bass_rust

    n_split = 0
    for fn in nc.m.functions:
        for bb in fn.blocks:
            out = []
            for ins in bb.instructions:
                si = ins.sync_info
                if si is not None and len(si.on_wait) > 1:
                    waits = list(si.on_wait)
                    for j, w in enumerate(waits[:-1]):
                        nop = bass_rust.InstNoOp(name=f"{ins.name}-sw{j}")
                        nop.engine = ins.engine
                        nop.sync_info = bass_rust.SyncInfo(
                            on_wait=[w], on_update=[])
                        out.append(nop)
                        n_split += 1
                    ins.sync_info = bass_rust.SyncInfo(
                        on_wait=[waits[-1]], on_update=list(si.on_update))
                out.append(ins)
            bb.instructions = out
    return n_split


def _get_nc():
    if "nc" not in _NC_CACHE:
        _NC_CACHE["nc"] = _build_nc()
    return _NC_CACHE["nc"]


def kernel(x, mean, std):
    x = np.asarray(x)
    if x.ndim == 3:
        x = x[:, 0, :]
    x = np.ascontiguousarray(x, dtype=np.float32)
    mean = np.ascontiguousarray(np.asarray(mean, np.float32).reshape(1, 3, 1, 1))
    std = np.ascontiguousarray(np.asarray(std, np.float32).reshape(1, 3, 1, 1))
    assert x.shape == (B_FULL, N), x.shape

    from concourse.bass_utils import run_bass_kernel_spmd

    nc = _get_nc()
    in_maps = [
        {"x": np.ascontiguousarray(x[i * B_LOC:(i + 1) * B_LOC]),
         "mean": mean, "std": std}
        for i in range(N_CORES)
    ]
    res = run_bass_kernel_spmd(nc, in_maps, list(range(N_CORES)))
    return np.concatenate(
        [res.results[i]["out"] for i in range(N_CORES)], axis=0)


if __name__ == "__main__":
    consts = build_consts()
    print("TOTD =", consts["totd"])
    print("tmask nonzero per jc:",
          [sum(1 for t in range(NT) if consts["tmask"][t][jc])
           for jc in range(2)])
